# Initial kernel scaffold
#
"""Trainium2 Bass kernel for nn_ChamferDistanceL2.

Math notes (exact reformulation of the reference):
  probs = softmax(logits) over V; the chamfer "y" cloud is one-hot rows of
  targets (masked), so the pairwise squared distances collapse to
      d2[b,i,j] = xs_i + mask_j - 2*mask_i*mask_j*probs[b,i,t_j]
  with xs_i = mask_i * sum_{v>=1} probs[b,i,v]^2.  Everything the device
  needs from the full [B,S,V] logits is:
      s_i = sum_v exp(l)        (ACT exp pass, accum)
      q_i = sum_v exp(2l)       (ACT exp pass, scale=2, accum)
      e0_i = exp(l[...,0]), and the gathered raw logits l[b,i,t_j].
  The gather indices/masks are pure functions of the tiny `targets` input,
  so the host precomputes them and the device does all the FLOPs.
"""

import os
import sys

sys.path.insert(0, "/opt/trn_rl_repo")

import numpy as np

B, S, V = 64, 128, 4096
M = 8                 # NeuronCores (data-parallel over batch)
BC = B // M           # batch elements per core
R = BC * S            # rows per core
EOS, PAD, EPS = 0, 4096, 1e-8
NEG = np.float32(-1e30)

_CACHE = {}


def _build_nc():
    import concourse.bass as bass
    import concourse.mybir as mybir
    from concourse.tile import TileContext
    from concourse.masks import make_identity

    f32 = mybir.dt.float32
    A = mybir.AluOpType
    AF = mybir.ActivationFunctionType
    X = mybir.AxisListType.X

    nc = bass.Bass()
    lgt = nc.dram_tensor("lgt", [R, V], f32, kind="ExternalInput")
    lge = nc.dram_tensor("lge", [R, S + 1], f32, kind="ExternalInput")
    mjb = nc.dram_tensor("mjb", [R, S], f32, kind="ExternalInput")
    msk = nc.dram_tensor("msk", [R, 3], f32, kind="ExternalInput")
    out = nc.dram_tensor("out", [4, BC], f32, kind="ExternalOutput")

    with TileContext(nc) as tc:
        with (
            tc.tile_pool(name="big", bufs=4) as bigp,
            tc.tile_pool(name="scr", bufs=2) as scrp,
            tc.tile_pool(name="sm", bufs=3) as smp,
            tc.tile_pool(name="keep", bufs=1) as keepp,
            tc.tile_pool(name="ps", bufs=2, space="PSUM") as psp,
            tc.tile_pool(name="ps2", bufs=2, space="PSUM") as psp2,
        ):
            ident = keepp.tile([128, 128], f32, tag="ident")
            make_identity(nc, ident[:])
            ones = keepp.tile([128, 1], f32, tag="ones")
            nc.vector.memset(ones[:], 1.0)

            sall = keepp.tile([128, BC], f32, tag="sall")     # softmax denoms
            omall = keepp.tile([128, BC], f32, tag="omall")   # 1 - p0
            l0s = keepp.tile([128, BC], f32, tag="l0s")       # raw eos logits
            eps_t = keepp.tile([128, BC], f32, tag="eps")     # eos_pos mask
            mhs = keepp.tile([128, BC], f32, tag="mhs")       # eos_head mask
            packs = keepp.tile([128, 4 * BC], f32, tag="packs")
            out_sb = keepp.tile([4, BC], f32, tag="outsb")

            # ---------------- phase 1: per-batch-element heavy work ----------
            for b in range(BC):
                rows = slice(b * 128, (b + 1) * 128)
                t_lgt = bigp.tile([128, V], f32, tag="lgt")
                for c in range(4):
                    cols = slice(c * (V // 4), (c + 1) * (V // 4))
                    nc.sync.dma_start(out=t_lgt[:, cols], in_=lgt[rows, cols])
                t_lge = smp.tile([128, S + 1], f32, tag="lge")
                nc.sync.dma_start(out=t_lge[:, :], in_=lge[rows, :])
                t_mjb = smp.tile([128, S], f32, tag="mjb")
                nc.sync.dma_start(out=t_mjb[:, :], in_=mjb[rows, :])
                t_msk = smp.tile([128, 3], f32, tag="msk")
                nc.sync.dma_start(out=t_msk[:, :], in_=msk[rows, :])

                mh = t_msk[:, 0:1]

                # ACT: the two exp passes over the full [128, V] tile
                scr1 = scrp.tile([128, V], f32, tag="scr")
                nc.scalar.activation(
                    scr1[:], t_lgt[:], AF.Exp, accum_out=sall[:, b : b + 1]
                )
                q = smp.tile([128, 1], f32, tag="q")
                scr2 = scrp.tile([128, V], f32, tag="scr")
                nc.scalar.activation(
                    scr2[:], t_lgt[:], AF.Exp, scale=2.0, accum_out=q[:]
                )
                # ACT: exp of the gathered logits (+ eos column S)
                eg = smp.tile([128, S + 1], f32, tag="eg")
                nc.scalar.activation(eg[:], t_lge[:], AF.Exp)

                # stash raw l0 and masks for phase 2
                nc.vector.tensor_copy(l0s[:, b : b + 1], t_lge[:, S : S + 1])
                nc.vector.tensor_copy(eps_t[:, b : b + 1], t_msk[:, 2:3])
                nc.vector.tensor_copy(mhs[:, b : b + 1], mh)

                # DVE: per-row softmax stats
                rs = smp.tile([128, 1], f32, tag="rs")
                nc.vector.reciprocal(rs[:], sall[:, b : b + 1])
                e0 = eg[:, S : S + 1]
                e0sq = smp.tile([128, 1], f32, tag="e0sq")
                nc.vector.tensor_mul(e0sq[:], e0, e0)
                qm = smp.tile([128, 1], f32, tag="qm")
                nc.vector.tensor_sub(qm[:], q[:], e0sq[:])
                rs2 = smp.tile([128, 1], f32, tag="rs2")
                nc.vector.tensor_mul(rs2[:], rs[:], rs[:])
                rs2m = smp.tile([128, 1], f32, tag="rs2m")
                nc.vector.tensor_mul(rs2m[:], rs2[:], mh)
                xs = smp.tile([128, 1], f32, tag="xs")
                nc.vector.tensor_mul(xs[:], qm[:], rs2m[:])
                m2rs = smp.tile([128, 1], f32, tag="m2rs")
                nc.vector.tensor_scalar(m2rs[:], rs[:], -2.0, None, A.mult)
                m2rsm = smp.tile([128, 1], f32, tag="m2rsm")
                nc.vector.tensor_mul(m2rsm[:], m2rs[:], mh)
                # p0 and 1-p0 for the BCE (Ln deferred to phase 2)
                p0 = smp.tile([128, 1], f32, tag="p0")
                nc.vector.tensor_mul(p0[:], e0, rs[:])
                nc.vector.tensor_scalar(
                    omall[:, b : b + 1], p0[:], -1.0, 1.0, A.mult, A.add
                )

                # DVE: chamfer distance matrix and its two mins
                d2a = smp.tile([128, S], f32, tag="d2a")
                nc.vector.tensor_scalar(
                    d2a[:], eg[:, 0:S], m2rsm[:], xs[:], A.mult, A.add
                )
                d2 = smp.tile([128, S], f32, tag="d2")
                nc.vector.tensor_add(d2[:], d2a[:], t_mjb[:])
                nc.vector.tensor_reduce(
                    packs[:, 4 * b : 4 * b + 1], d2[:], axis=X, op=A.min
                )
                pt = psp.tile([128, 128], f32, tag="pt")
                nc.tensor.transpose(pt[:], d2[:], ident[:])
                nc.vector.tensor_reduce(
                    packs[:, 4 * b + 1 : 4 * b + 2], pt[:], axis=X, op=A.min
                )

            # ---------------- phase 2: BCE (one Exp->Ln table switch) --------
            lns = keepp.tile([128, BC], f32, tag="lns")
            nc.scalar.activation(lns[:], sall[:], AF.Ln)
            lom = keepp.tile([128, BC], f32, tag="lom")
            nc.scalar.activation(lom[:], omall[:], AF.Ln)

            logp = keepp.tile([128, BC], f32, tag="logp")
            nc.vector.tensor_sub(logp[:], l0s[:], lns[:])
            nc.vector.tensor_scalar_max(logp[:], logp[:], -100.0)
            nc.vector.tensor_scalar_max(lom[:], lom[:], -100.0)
            ets = keepp.tile([128, BC], f32, tag="ets")
            nc.vector.tensor_scalar(ets[:], mhs[:], -1.0, 1.0, A.mult, A.add)

            for b in range(BC):
                col = slice(b, b + 1)
                u = smp.tile([128, 1], f32, tag="u")
                nc.vector.select(
                    u[:], ets[:, col], logp[:, col], lom[:, col]
                )
                nc.vector.tensor_mul(
                    packs[:, 4 * b + 2 : 4 * b + 3], u[:], eps_t[:, col]
                )
                nc.vector.tensor_mul(
                    packs[:, 4 * b + 3 : 4 * b + 4], u[:], mhs[:, col]
                )
                psum = psp2.tile([4, 1], f32, tag="psum")
                nc.tensor.matmul(
                    psum[:], lhsT=packs[:, 4 * b : 4 * b + 4], rhs=ones[:],
                    start=True, stop=True,
                )
                nc.vector.tensor_copy(out_sb[:, b : b + 1], psum[:])

            nc.sync.dma_start(out=out[:, :], in_=out_sb[:, :])

    return nc


def _get_nc():
    if "nc" not in _CACHE:
        _CACHE["nc"] = _build_nc()
    return _CACHE["nc"]


def _prep(logits, targets):
    """Host-side prep: masks, counts, gathered raw logits (all from the tiny
    `targets` tensor + a 4MB fancy-index into logits)."""
    logits = np.ascontiguousarray(np.asarray(logits, dtype=np.float32))
    t = np.asarray(targets).astype(np.int64)
    mh = ((t != PAD) & (t != EOS)).astype(np.float32)   # eos_head
    ep = (t == EOS).astype(np.float32)                  # eos_pos
    cep, ceh = ep.sum(1), mh.sum(1)
    tclip = np.minimum(t, V - 1)
    lg = np.take_along_axis(
        logits, np.broadcast_to(tclip[:, None, :], (B, S, S)), axis=2
    )
    lgm = np.where(mh[:, None, :] > 0, lg, NEG)
    lge = np.concatenate([lgm, logits[:, :, 0:1]], axis=2)       # [B,S,S+1]
    mjb = np.broadcast_to(mh[:, None, :], (B, S, S))             # [B,S,S]
    msk = np.stack([mh, 1.0 - mh, ep], axis=2)                   # [B,S,3]
    return logits, lge, mjb, msk, cep, ceh


def _combine(outs, cep, ceh):
    """outs: [M][4, BC] per-core partial sums -> final [2] float32."""
    o = np.stack([np.asarray(x) for x in outs])        # [M, 4, BC]
    sum0 = o[:, 0, :].reshape(-1)                      # [B] sum_i min_j d2
    sum1 = o[:, 1, :].reshape(-1)                      # [B] sum_j min_i d2
    uep = o[:, 2, :].reshape(-1)
    ueh = o[:, 3, :].reshape(-1)
    label = np.mean((sum0 + sum1) / S)
    eos = np.mean(0.5 * (-uep) / (cep + EPS) + 0.5 * (-ueh) / (ceh + EPS))
    return np.stack([label, eos]).astype(np.float32)


def _in_maps(logits, lge, mjb, msk):
    maps = []
    for c in range(M):
        bs = slice(c * BC, (c + 1) * BC)
        maps.append(
            {
                "lgt": np.ascontiguousarray(logits[bs].reshape(R, V)),
                "lge": np.ascontiguousarray(lge[bs].reshape(R, S + 1)),
                "mjb": np.ascontiguousarray(mjb[bs].reshape(R, S)),
                "msk": np.ascontiguousarray(msk[bs].reshape(R, 3)),
            }
        )
    return maps


def kernel(logits, targets):
    logits, lge, mjb, msk, cep, ceh = _prep(logits, targets)
    maps = _in_maps(logits, lge, mjb, msk)
    nc = _get_nc()

    if os.environ.get("KMODE") == "sim":
        from concourse import bass_interp

        outs = []
        for c in range(M):
            sim = bass_interp.CoreSim(nc)
            for k, v in maps[c].items():
                sim.tensor(k)[:] = v
            sim.simulate()
            outs.append(np.array(sim.tensor("out")))
    else:
        from concourse.bass_utils import run_bass_kernel_spmd

        res = run_bass_kernel_spmd(nc, maps, list(range(M)))
        outs = [res.results[c]["out"] for c in range(M)]

    return _combine(outs, cep, ceh)


# revision 10
# speedup vs baseline: 1.0930x; 1.0930x over previous
"""Trainium2 Bass kernel for nn_ChamferDistanceL2.

Math notes (exact reformulation of the reference):
  probs = softmax(logits) over V; the chamfer "y" cloud is one-hot rows of
  targets (masked), so the pairwise squared distances collapse to
      d2[b,i,j] = xs_i + mask_j - 2*mask_i*mask_j*probs[b,i,t_j]
  with xs_i = mask_i * sum_{v>=1} probs[b,i,v]^2.  Everything the device
  needs from the full [B,S,V] logits is:
      s_i = sum_v exp(l)        (ACT exp pass, accum)
      q_i = sum_v exp(2l)       (ACT exp pass, scale=2, accum)
      e0_i = exp(l[...,0]), and the gathered raw logits l[b,i,t_j].
  The gather indices/masks are pure functions of the tiny `targets` input,
  so the host precomputes them and the device does all the FLOPs.
"""

import os
import sys

sys.path.insert(0, "/opt/trn_rl_repo")

import numpy as np

B, S, V = 64, 128, 4096
M = 8                 # NeuronCores (data-parallel over batch)
BC = B // M           # batch elements per core
R = BC * S            # rows per core
EOS, PAD, EPS = 0, 4096, 1e-8
NEG = np.float32(-1e30)

_CACHE = {}


def _build_nc(reps=1):
    import concourse.bacc as bacc
    import concourse.mybir as mybir
    from concourse.tile import TileContext
    from concourse.masks import make_identity

    f32 = mybir.dt.float32
    A = mybir.AluOpType
    AF = mybir.ActivationFunctionType
    X = mybir.AxisListType.X

    nc = bacc.Bacc()
    lgt = nc.dram_tensor("lgt", [R, V], f32, kind="ExternalInput")
    lge = nc.dram_tensor("lge", [R, S + 1], f32, kind="ExternalInput")
    mjb = nc.dram_tensor("mjb", [R, S], f32, kind="ExternalInput")
    msk = nc.dram_tensor("msk", [R, 3], f32, kind="ExternalInput")
    out = nc.dram_tensor("out", [4, BC], f32, kind="ExternalOutput")

    with TileContext(nc) as tc:
        with (
            tc.tile_pool(name="big", bufs=4) as bigp,
            tc.tile_pool(name="scr", bufs=2) as scrp,
            tc.tile_pool(name="sm", bufs=3) as smp,
            tc.tile_pool(name="keep", bufs=1) as keepp,
            tc.tile_pool(name="ps", bufs=2, space="PSUM") as psp,
            tc.tile_pool(name="ps2", bufs=2, space="PSUM") as psp2,
        ):
            ident = keepp.tile([128, 128], f32, tag="ident")
            make_identity(nc, ident[:])
            ones = keepp.tile([128, 1], f32, tag="ones")
            nc.vector.memset(ones[:], 1.0)

            sall = keepp.tile([128, BC], f32, tag="sall")     # softmax denoms
            omall = keepp.tile([128, BC], f32, tag="omall")   # 1 - p0
            l0s = keepp.tile([128, BC], f32, tag="l0s")       # raw eos logits
            eps_t = keepp.tile([128, BC], f32, tag="eps")     # eos_pos mask
            mhs = keepp.tile([128, BC], f32, tag="mhs")       # eos_head mask
            packs = keepp.tile([128, 4 * BC], f32, tag="packs")
            out_sb = keepp.tile([4, BC], f32, tag="outsb")

            # reps>1 repeats the computation for marginal-time benchmarking
            for _rep in range(reps):
                # ------------- phase 1: per-batch-element heavy work ---------
                for b in range(BC):
                    rows = slice(b * 128, (b + 1) * 128)
                    t_lgt = bigp.tile([128, V], f32, tag="lgt")
                    nc.sync.dma_start(out=t_lgt[:, :], in_=lgt[rows, :])
                    t_lge = smp.tile([128, S + 1], f32, tag="lge")
                    nc.sync.dma_start(out=t_lge[:, :], in_=lge[rows, :])
                    t_mjb = smp.tile([128, S], f32, tag="mjb")
                    nc.sync.dma_start(out=t_mjb[:, :], in_=mjb[rows, :])
                    t_msk = smp.tile([128, 3], f32, tag="msk")
                    nc.sync.dma_start(out=t_msk[:, :], in_=msk[rows, :])

                    mh = t_msk[:, 0:1]

                    # ACT: the two exp passes over the full [128, V] tile
                    scr1 = scrp.tile([128, V], f32, tag="scr")
                    nc.scalar.activation(
                        scr1[:], t_lgt[:], AF.Exp, accum_out=sall[:, b : b + 1]
                    )
                    q = smp.tile([128, 1], f32, tag="q")
                    scr2 = scrp.tile([128, V], f32, tag="scr")
                    nc.scalar.activation(
                        scr2[:], t_lgt[:], AF.Exp, scale=2.0, accum_out=q[:]
                    )
                    # ACT: exp of the gathered logits (+ eos column S)
                    eg = smp.tile([128, S + 1], f32, tag="eg")
                    nc.scalar.activation(eg[:], t_lge[:], AF.Exp)

                    # stash raw l0 and masks for phase 2
                    nc.vector.tensor_copy(l0s[:, b : b + 1], t_lge[:, S : S + 1])
                    nc.vector.tensor_copy(eps_t[:, b : b + 1], t_msk[:, 2:3])
                    nc.vector.tensor_copy(mhs[:, b : b + 1], mh)

                    # DVE: per-row softmax stats
                    rs = smp.tile([128, 1], f32, tag="rs")
                    nc.vector.reciprocal(rs[:], sall[:, b : b + 1])
                    e0 = eg[:, S : S + 1]
                    e0sq = smp.tile([128, 1], f32, tag="e0sq")
                    nc.vector.tensor_mul(e0sq[:], e0, e0)
                    qm = smp.tile([128, 1], f32, tag="qm")
                    nc.vector.tensor_sub(qm[:], q[:], e0sq[:])
                    rs2 = smp.tile([128, 1], f32, tag="rs2")
                    nc.vector.tensor_mul(rs2[:], rs[:], rs[:])
                    rs2m = smp.tile([128, 1], f32, tag="rs2m")
                    nc.vector.tensor_mul(rs2m[:], rs2[:], mh)
                    xs = smp.tile([128, 1], f32, tag="xs")
                    nc.vector.tensor_mul(xs[:], qm[:], rs2m[:])
                    m2rs = smp.tile([128, 1], f32, tag="m2rs")
                    nc.vector.tensor_scalar(m2rs[:], rs[:], -2.0, None, A.mult)
                    m2rsm = smp.tile([128, 1], f32, tag="m2rsm")
                    nc.vector.tensor_mul(m2rsm[:], m2rs[:], mh)
                    # p0 and 1-p0 for the BCE (Ln deferred to phase 2)
                    p0 = smp.tile([128, 1], f32, tag="p0")
                    nc.vector.tensor_mul(p0[:], e0, rs[:])
                    nc.vector.tensor_scalar(
                        omall[:, b : b + 1], p0[:], -1.0, 1.0, A.mult, A.add
                    )

                    # DVE: chamfer distance matrix and its two mins
                    d2a = smp.tile([128, S], f32, tag="d2a")
                    nc.vector.tensor_scalar(
                        d2a[:], eg[:, 0:S], m2rsm[:], xs[:], A.mult, A.add
                    )
                    d2 = smp.tile([128, S], f32, tag="d2")
                    nc.vector.tensor_add(d2[:], d2a[:], t_mjb[:])
                    nc.vector.tensor_reduce(
                        packs[:, 4 * b : 4 * b + 1], d2[:], axis=X, op=A.min
                    )
                    pt = psp.tile([128, 128], f32, tag="pt")
                    nc.tensor.transpose(pt[:], d2[:], ident[:])
                    nc.vector.tensor_reduce(
                        packs[:, 4 * b + 1 : 4 * b + 2], pt[:], axis=X, op=A.min
                    )

                # ------------- phase 2: BCE (one Exp->Ln table switch) -------
                lns = keepp.tile([128, BC], f32, tag="lns")
                nc.scalar.activation(lns[:], sall[:], AF.Ln)
                lom = keepp.tile([128, BC], f32, tag="lom")
                nc.scalar.activation(lom[:], omall[:], AF.Ln)

                # bce = -(t*log_p + (1-t)*log_1mp); the two reduced terms only
                # ever see t=1 rows (ep subset) and t=0 rows (eh subset), so
                # the select collapses: sum(bce*ep) = -sum(logp*ep),
                # sum(bce*eh) = -sum(lom*eh).
                logp = keepp.tile([128, BC], f32, tag="logp")
                nc.vector.tensor_sub(logp[:], l0s[:], lns[:])
                nc.vector.tensor_scalar_max(logp[:], logp[:], -100.0)
                nc.vector.tensor_scalar_max(lom[:], lom[:], -100.0)

                for b in range(BC):
                    col = slice(b, b + 1)
                    nc.vector.tensor_mul(
                        packs[:, 4 * b + 2 : 4 * b + 3], logp[:, col], eps_t[:, col]
                    )
                    nc.vector.tensor_mul(
                        packs[:, 4 * b + 3 : 4 * b + 4], lom[:, col], mhs[:, col]
                    )
                    psum = psp2.tile([4, 1], f32, tag="psum")
                    nc.tensor.matmul(
                        psum[:], lhsT=packs[:, 4 * b : 4 * b + 4], rhs=ones[:],
                        start=True, stop=True,
                    )
                    nc.vector.tensor_copy(out_sb[:, b : b + 1], psum[:])

                nc.sync.dma_start(out=out[:, :], in_=out_sb[:, :])

    nc.compile()
    return nc


def _get_nc():
    if "nc" not in _CACHE:
        _CACHE["nc"] = _build_nc()
    return _CACHE["nc"]


def _prep(logits, targets):
    """Host-side prep: masks, counts, gathered raw logits (all from the tiny
    `targets` tensor + a 4MB fancy-index into logits)."""
    logits = np.ascontiguousarray(np.asarray(logits, dtype=np.float32))
    t = np.asarray(targets).astype(np.int64)
    mh = ((t != PAD) & (t != EOS)).astype(np.float32)   # eos_head
    ep = (t == EOS).astype(np.float32)                  # eos_pos
    cep, ceh = ep.sum(1), mh.sum(1)
    tclip = np.minimum(t, V - 1)
    lg = np.take_along_axis(
        logits, np.broadcast_to(tclip[:, None, :], (B, S, S)), axis=2
    )
    lgm = np.where(mh[:, None, :] > 0, lg, NEG)
    lge = np.concatenate([lgm, logits[:, :, 0:1]], axis=2)       # [B,S,S+1]
    mjb = np.broadcast_to(mh[:, None, :], (B, S, S))             # [B,S,S]
    msk = np.stack([mh, 1.0 - mh, ep], axis=2)                   # [B,S,3]
    return logits, lge, mjb, msk, cep, ceh


def _combine(outs, cep, ceh):
    """outs: [M][4, BC] per-core partial sums -> final [2] float32."""
    o = np.stack([np.asarray(x) for x in outs])        # [M, 4, BC]
    sum0 = o[:, 0, :].reshape(-1)                      # [B] sum_i min_j d2
    sum1 = o[:, 1, :].reshape(-1)                      # [B] sum_j min_i d2
    uep = o[:, 2, :].reshape(-1)
    ueh = o[:, 3, :].reshape(-1)
    label = np.mean((sum0 + sum1) / S)
    eos = np.mean(0.5 * (-uep) / (cep + EPS) + 0.5 * (-ueh) / (ceh + EPS))
    return np.stack([label, eos]).astype(np.float32)


def _in_maps(logits, lge, mjb, msk):
    maps = []
    for c in range(M):
        bs = slice(c * BC, (c + 1) * BC)
        maps.append(
            {
                "lgt": np.ascontiguousarray(logits[bs].reshape(R, V)),
                "lge": np.ascontiguousarray(lge[bs].reshape(R, S + 1)),
                "mjb": np.ascontiguousarray(mjb[bs].reshape(R, S)),
                "msk": np.ascontiguousarray(msk[bs].reshape(R, 3)),
            }
        )
    return maps


def kernel(logits, targets):
    logits, lge, mjb, msk, cep, ceh = _prep(logits, targets)
    maps = _in_maps(logits, lge, mjb, msk)
    nc = _get_nc()

    if os.environ.get("KMODE") == "sim":
        from concourse import bass_interp

        outs = []
        for c in range(M):
            sim = bass_interp.CoreSim(nc)
            for k, v in maps[c].items():
                sim.tensor(k)[:] = v
            sim.simulate()
            outs.append(np.array(sim.tensor("out")))
    else:
        from concourse.bass_utils import run_bass_kernel_spmd

        res = run_bass_kernel_spmd(nc, maps, list(range(M)))
        outs = [res.results[c]["out"] for c in range(M)]

    return _combine(outs, cep, ceh)
